# revision 2
# baseline (speedup 1.0000x reference)
"""DynamicGraphEmbedding kernel for 8 Trainium2 NeuronCores.

The reference collapses algebraically:
  - deg[i] == K == 16 for every node (dst list is repeat(arange(N), K)),
    so gcn_norm edge weight ew == 1/16 for every edge.
  - straight-through gumbel gate is exactly y_hard in the forward pass,
    i.e. gate(e) = 1 iff argmax(softmax(logits[e] + g[e])) == 0.
  - therefore out[b] = A @ (x[b] @ W) + bias, with the dense [N, N] matrix
    A[i, j] = gate(i*N+j)/16 if j in topk_j[i] else 0.

Host (tiny, O(N^2)): build A from emb/logits/gumbel_u with the exact same
jax-on-CPU ops as the reference. Device (the memory-bound bulk): two chained
256^3 matmuls per batch element, data-parallel over batch across 8 cores.

All device traffic and matmuls are fp16 (halves HBM bytes vs f32 and enables
the PE fast-weight-load path); PSUM accumulation stays f32, output is
upcast to f32 on the host. A entries are gate/16, exact in fp16, so the only
quantization is x/W/h/out rounding: ~1e-3 relative, well under tolerance.
"""

import sys

import numpy as np

if "/opt/trn_rl_repo" not in sys.path:
    sys.path.insert(0, "/opt/trn_rl_repo")

N, T, B, D, K = 256, 256, 64, 64, 16
NCORES = 8
BPC = B // NCORES  # batch elements per core
NG = BPC // 2  # batch pairs per core

_CACHE = {}
LAST_RESULT = None  # BassKernelResults of the most recent run (for profiling)


def _graph_matrix(emb, logits, gumbel_u):
    """Dense [N, N] combined gate/topk/gcn-norm matrix A (host-side, tiny)."""
    try:
        import jax
        import jax.numpy as jnp

        cpu = jax.devices("cpu")[0]
        emb_j = jax.device_put(np.asarray(emb), cpu)
        logits_j = jax.device_put(np.asarray(logits), cpu)
        gu_j = jax.device_put(np.asarray(gumbel_u), cpu)
        nrm = jnp.linalg.norm(emb_j, axis=-1)
        cos = (emb_j @ emb_j.T) / (nrm[:, None] * nrm[None, :])
        _, topk_j = jax.lax.top_k(cos, K)
        g = -jnp.log(-jnp.log(gu_j))
        y_soft = jax.nn.softmax(logits_j + g, axis=-1)
        am = jnp.argmax(y_soft, axis=-1)
        topk = np.asarray(topk_j)
        gate_full = (np.asarray(am) == 0).astype(np.float32)
    except Exception:
        emb32 = np.asarray(emb, np.float32)
        nrm = np.sqrt((emb32 * emb32).sum(-1))
        cos = (emb32 @ emb32.T) / (nrm[:, None] * nrm[None, :])
        topk = np.argsort(-cos, axis=-1, kind="stable")[:, :K]
        lg = np.asarray(logits, np.float32) + np.float32(-1.0) * np.log(
            -np.log(np.asarray(gumbel_u, np.float32))
        )
        e = np.exp(lg - lg.max(-1, keepdims=True))
        y_soft = e / e.sum(-1, keepdims=True)
        gate_full = (np.argmax(y_soft, -1) == 0).astype(np.float32)
    rows = np.repeat(np.arange(N), K)
    cols = topk.reshape(-1)
    A = np.zeros((N, N), np.float32)
    A[rows, cols] = gate_full[rows * N + cols] * np.float32(0.0625)
    return A


def _build_bass(with_bias):
    """Per-core Bass graph: out[b] = A @ (x[b] @ W) [+ bias] for BPC batches.

    Host-packed fp16 layouts (contiguous per-partition runs, few big DMAs):
      consts [128, 4, 256]        [p, g, t]: g = (W c0, W c1, AT c0, AT c1)
      xin    [NG, 128, 2, 2, 256] [g, p, c, bi, n] = x[2g+bi][n, c*128+p]
      bias   [1, 256]             (only when with_bias)
      outp   [NG, 128, 2, 2, 256] [g, p, m, bi, t] = out[2g+bi][m*128+p, t]
    """
    import concourse.bass as bass
    import concourse.mybir as mybir
    from concourse import bacc
    from concourse.tile import TileContext

    F32 = mybir.dt.float32
    F16 = mybir.dt.float16

    nc = bacc.Bacc()
    consts = nc.declare_dram_parameter("consts", [128, 4, 256], F16, isOutput=False)
    xin = nc.declare_dram_parameter("xin", [NG, 128, 2, 2, N], F16, isOutput=False)
    if with_bias:
        bp = nc.declare_dram_parameter("bias", [1, T], F32, isOutput=False)
    outp = nc.declare_dram_parameter("outp", [NG, 128, 2, 2, T], F16, isOutput=True)

    with TileContext(nc) as tc:
        with (
            tc.tile_pool(name="const", bufs=1) as const,
            tc.tile_pool(name="xpool", bufs=4) as xpool,
            tc.tile_pool(name="hbuf", bufs=3) as hbuf,
            tc.tile_pool(name="obuf", bufs=3) as obuf,
            tc.tile_pool(name="psA", bufs=4, space="PSUM") as psA,
            tc.tile_pool(name="psB", bufs=2, space="PSUM") as psB,
            tc.tile_pool(name="psW", bufs=1, space="PSUM") as psW,
        ):
            ct = const.tile([128, 4, 256], F16)
            # Loads in critical-path order: W chunks -> x pair 0 (split
            # across both hwdge queues) -> AT chunks -> remaining x pairs.
            nc.sync.dma_start(out=ct[:, 0:2, :], in_=consts[:, 0:2, :])
            if with_bias:
                bias_bc = const.tile([128, T], F32)
                nc.gpsimd.dma_start(out=bias_bc, in_=bp.ap().to_broadcast([128, T]))

            # Pre-warm the PE HAM clock gate during the initial loads: a
            # memset-fed dummy matmul stream keeps PE busy so the real
            # matmuls ramp toward 2.4 GHz. Sized to end roughly when the
            # first x pair lands (~1.5us after the queue drains the loads).
            scratch = const.tile([128, 512], F16, tag="warm")
            nc.vector.memset(scratch, 0.0)
            wps = psW.tile([128, 512], F32)
            for _ in range(3):
                nc.tensor.matmul(
                    wps,
                    lhsT=scratch[:, 0:128],
                    rhs=scratch,
                    start=True,
                    stop=True,
                )

            xts = []
            for g in range(NG):
                xt = xpool.tile([128, 2, 2, N], F16)  # [p=t%128, c, bi, n]
                if g == 0:
                    # split the critical first pair across both hwdge
                    # engines so its two halves stream concurrently
                    nc.sync.dma_start(out=xt[:, 0, :, :], in_=xin[g][:, 0])
                    nc.scalar.dma_start(out=xt[:, 1, :, :], in_=xin[g][:, 1])
                else:
                    nc.sync.dma_start(out=xt, in_=xin[g])
                xts.append(xt)
                if g == 0:
                    nc.sync.dma_start(out=ct[:, 2:4, :], in_=consts[:, 2:4, :])

            for g in range(NG):
                xt = xts[g]
                # h for the pair: [p=j%128, jc(=node block m), bi, t']
                h_sb = hbuf.tile([128, 2, 2, T], F16)
                for bi in range(2):
                    for m in range(2):
                        ph = psA.tile([128, T], F32)
                        nc.tensor.matmul(
                            ph,
                            lhsT=xt[:, 0, bi, bass.ts(m, 128)],
                            rhs=ct[:, 0, :],
                            start=True,
                            stop=False,
                        )
                        nc.tensor.matmul(
                            ph,
                            lhsT=xt[:, 1, bi, bass.ts(m, 128)],
                            rhs=ct[:, 1, :],
                            start=False,
                            stop=True,
                        )
                        nc.vector.tensor_copy(h_sb[:, m, bi, :], ph)
                ob = obuf.tile([128, 2, 2, T], F16)  # [p=i%128, m, bi, t']
                for m in range(2):
                    po = psB.tile([128, 2, T], F32)  # [i%128, bi, t'] one bank
                    nc.tensor.matmul(
                        po,
                        lhsT=ct[:, 2, bass.ts(m, 128)],
                        rhs=h_sb[:, 0, :, :],
                        start=True,
                        stop=False,
                    )
                    nc.tensor.matmul(
                        po,
                        lhsT=ct[:, 3, bass.ts(m, 128)],
                        rhs=h_sb[:, 1, :, :],
                        start=False,
                        stop=True,
                    )
                    last = g == NG - 1
                    if with_bias:
                        for bi in range(2):
                            nc.vector.tensor_add(
                                ob[:, m, bi, :], po[:, bi, :], bias_bc
                            )
                    elif last:
                        # tail: DVE is idle and faster than ACT here
                        nc.vector.tensor_copy(ob[:, m], po)
                    else:
                        # staging copy on ACT keeps DVE free for the h copies
                        nc.scalar.copy(out=ob[:, m], in_=po)
                if g == NG - 1:
                    # split the final store across both hwdge queues to
                    # halve the end-of-kernel drain
                    nc.sync.dma_start(out=outp[g][:, 0], in_=ob[:, 0])
                    nc.scalar.dma_start(out=outp[g][:, 1], in_=ob[:, 1])
                else:
                    # SWDGE queue: keeps stores off the load/copy engines
                    nc.gpsimd.dma_start(out=outp[g], in_=ob)
    nc.finalize()
    return nc


def _ensure_axon_hooks_importable():
    """concourse's trace path hard-imports antenv.axon_hooks, which this
    image lacks. Provide the real ctypes-backed hook when possible, else a
    no-op, so BASS_TRACE=1 degrades gracefully instead of crashing."""
    try:
        import antenv.axon_hooks  # noqa: F401

        return
    except ImportError:
        pass
    try:
        import types

        import antenv

        mod = types.ModuleType("antenv.axon_hooks")
        state = {"h": None}
        mod.set_axon_ntff_profile_hook = lambda h: state.__setitem__("h", h)
        mod.get_axon_ntff_profile_hook = lambda: state["h"]
        sys.modules["antenv.axon_hooks"] = mod
        antenv.axon_hooks = mod
        try:
            from trn_agent_boot.trn_boot import _ntff_profile_via_ctypes

            hook = _ntff_profile_via_ctypes("/opt/axon/libaxon_pjrt.so")
            if hook is not None:
                mod.set_axon_ntff_profile_hook(hook)
        except Exception:
            pass
    except Exception:
        pass


def kernel(x, emb, W, b, logits, gumbel_u):
    global LAST_RESULT
    _ensure_axon_hooks_importable()
    from concourse.bass_utils import run_bass_kernel_spmd

    x = np.asarray(x, np.float32)
    W = np.asarray(W, np.float32)
    bias = np.ascontiguousarray(np.asarray(b, np.float32)).reshape(1, T)

    A = _graph_matrix(emb, logits, gumbel_u)
    # consts [128, 4, 256]: W chunks then AT chunks, partition-major
    Wr = W.reshape(2, 128, T).transpose(1, 0, 2)
    Ar = np.ascontiguousarray(A.T).reshape(2, 128, N).transpose(1, 0, 2)
    consts = np.ascontiguousarray(
        np.concatenate([Wr, Ar], axis=1).astype(np.float16)
    )

    # xin [B/2 pairs, p, c, bi, n]: xT[b][t, n] split t = c*128+p, b = 2g+bi
    xT = x.transpose(0, 2, 1)  # [B, T, N]
    xpack = np.ascontiguousarray(
        xT.reshape(B // 2, 2, 2, 128, N).transpose(0, 3, 2, 1, 4).astype(np.float16)
    )

    with_bias = bool(np.any(bias))
    key = ("nc", with_bias)
    if key not in _CACHE:
        _CACHE[key] = _build_bass(with_bias)
    nc = _CACHE[key]

    in_maps = [
        {"xin": xpack[c * NG : (c + 1) * NG], "consts": consts}
        for c in range(NCORES)
    ]
    if with_bias:
        for m in in_maps:
            m["bias"] = bias
    # The first execution of a fresh NEFF occasionally trips a transient
    # NRT_EXEC_UNIT_UNRECOVERABLE; a straight retry reliably succeeds.
    last_exc = None
    for _attempt in range(3):
        try:
            res = run_bass_kernel_spmd(nc, in_maps, core_ids=list(range(NCORES)))
            break
        except Exception as e:  # noqa: BLE001
            last_exc = e
            import time as _time

            _time.sleep(2.0)
    else:
        raise last_exc
    LAST_RESULT = res
    # outp [NG, p, m, bi, t] -> out[2g+bi, m*128+p, t], upcast to f32
    out = np.empty((B, N, T), np.float32)
    for c in range(NCORES):
        ob = res.results[c]["outp"].astype(np.float32)  # [NG,128,2,2,T]
        ob = ob.transpose(0, 3, 2, 1, 4).reshape(BPC, N, T)  # [2g+bi, m*128+p, t]
        out[c * BPC : (c + 1) * BPC] = ob
    return out


# revision 4
# speedup vs baseline: 1.0829x; 1.0829x over previous
"""DynamicGraphEmbedding kernel for 8 Trainium2 NeuronCores.

The reference collapses algebraically:
  - deg[i] == K == 16 for every node (dst list is repeat(arange(N), K)),
    so gcn_norm edge weight ew == 1/16 for every edge.
  - straight-through gumbel gate is exactly y_hard in the forward pass,
    i.e. gate(e) = 1 iff argmax(softmax(logits[e] + g[e])) == 0.
  - therefore out[b] = A @ (x[b] @ W) + bias, with the dense [N, N] matrix
    A[i, j] = gate(i*N+j)/16 if j in topk_j[i] else 0.

Host (tiny, O(N^2)): build A from emb/logits/gumbel_u with the exact same
jax-on-CPU ops as the reference. Device (the memory-bound bulk): two chained
256^3 matmuls per batch element, data-parallel over batch across 8 cores.

All device traffic and matmuls are fp16 (halves HBM bytes vs f32 and enables
the PE fast-weight-load path); PSUM accumulation stays f32, output is
upcast to f32 on the host. A entries are gate/16, exact in fp16, so the only
quantization is x/W/h/out rounding: ~1e-3 relative, well under tolerance.
"""

import sys

import numpy as np

if "/opt/trn_rl_repo" not in sys.path:
    sys.path.insert(0, "/opt/trn_rl_repo")

N, T, B, D, K = 256, 256, 64, 64, 16
NCORES = 8
BPC = B // NCORES  # batch elements per core
NG = BPC // 2  # batch pairs per core
N_WARM = 6  # PE HAM warm-up matmuls (512-free each, ~0.43us cold)

_CACHE = {}
LAST_RESULT = None  # BassKernelResults of the most recent run (for profiling)


def _graph_matrix(emb, logits, gumbel_u):
    """Dense [N, N] combined gate/topk/gcn-norm matrix A (host-side, tiny)."""
    try:
        import jax
        import jax.numpy as jnp

        cpu = jax.devices("cpu")[0]
        emb_j = jax.device_put(np.asarray(emb), cpu)
        logits_j = jax.device_put(np.asarray(logits), cpu)
        gu_j = jax.device_put(np.asarray(gumbel_u), cpu)
        nrm = jnp.linalg.norm(emb_j, axis=-1)
        cos = (emb_j @ emb_j.T) / (nrm[:, None] * nrm[None, :])
        _, topk_j = jax.lax.top_k(cos, K)
        g = -jnp.log(-jnp.log(gu_j))
        y_soft = jax.nn.softmax(logits_j + g, axis=-1)
        am = jnp.argmax(y_soft, axis=-1)
        topk = np.asarray(topk_j)
        gate_full = (np.asarray(am) == 0).astype(np.float32)
    except Exception:
        emb32 = np.asarray(emb, np.float32)
        nrm = np.sqrt((emb32 * emb32).sum(-1))
        cos = (emb32 @ emb32.T) / (nrm[:, None] * nrm[None, :])
        topk = np.argsort(-cos, axis=-1, kind="stable")[:, :K]
        lg = np.asarray(logits, np.float32) + np.float32(-1.0) * np.log(
            -np.log(np.asarray(gumbel_u, np.float32))
        )
        e = np.exp(lg - lg.max(-1, keepdims=True))
        y_soft = e / e.sum(-1, keepdims=True)
        gate_full = (np.argmax(y_soft, -1) == 0).astype(np.float32)
    rows = np.repeat(np.arange(N), K)
    cols = topk.reshape(-1)
    A = np.zeros((N, N), np.float32)
    A[rows, cols] = gate_full[rows * N + cols] * np.float32(0.0625)
    return A


def _build_bass(with_bias):
    """Per-core Bass graph: out[b] = A @ (x[b] @ W) [+ bias] for BPC batches.

    Host-packed fp16 layouts (contiguous per-partition runs, few big DMAs):
      wmat   [128, 2, 256]        [p, c, t']: W[c*128+p, t']
      amat   [128, 2, 256]        [p, c, i]:  A[i, c*128+p]  (A^T chunks)
      xin    [NG, 128, 2, 2, 256] [g, p, c, bi, n] = x[2g+bi][n, c*128+p]
      bias   [1, 256]             (only when with_bias)
      outp   [NG, 128, 2, 2, 256] [g, p, m, bi, t] = out[2g+bi][m*128+p, t]
    """
    import concourse.bass as bass
    import concourse.mybir as mybir
    from concourse import bacc
    from concourse.tile import TileContext

    F32 = mybir.dt.float32
    F16 = mybir.dt.float16

    nc = bacc.Bacc()
    wmat = nc.declare_dram_parameter("wmat", [128, 2, 256], F16, isOutput=False)
    amat = nc.declare_dram_parameter("amat", [128, 2, 256], F16, isOutput=False)
    xin = nc.declare_dram_parameter("xin", [NG, 128, 2, 2, N], F16, isOutput=False)
    if with_bias:
        bp = nc.declare_dram_parameter("bias", [1, T], F32, isOutput=False)
    outp = nc.declare_dram_parameter("outp", [NG, 128, 2, 2, T], F16, isOutput=True)

    with TileContext(nc) as tc:
        with (
            tc.tile_pool(name="const", bufs=1) as const,
            tc.tile_pool(name="xpool", bufs=4) as xpool,
            tc.tile_pool(name="hbuf", bufs=3) as hbuf,
            tc.tile_pool(name="obuf", bufs=3) as obuf,
            tc.tile_pool(name="psA", bufs=4, space="PSUM") as psA,
            tc.tile_pool(name="psB", bufs=2, space="PSUM") as psB,
            tc.tile_pool(name="psW", bufs=1, space="PSUM") as psW,
        ):
            # Loads in critical-path order. First matmul needs x pair 0 + W:
            # x halves on the sync queue, W on the scalar queue so all three
            # stream concurrently from the first microsecond.
            wt = const.tile([128, 2, 256], F16)
            at = const.tile([128, 2, 256], F16)
            xts = []
            for g in range(NG):
                xt = xpool.tile([128, 2, 2, N], F16, tag=f"xt{g}")  # [p,c,bi,n]
                xts.append(xt)
            nc.sync.dma_start(out=xts[0][:, 0], in_=xin[0][:, 0])
            nc.scalar.dma_start(out=wt, in_=wmat.ap())
            nc.sync.dma_start(out=xts[0][:, 1], in_=xin[0][:, 1])
            nc.scalar.dma_start(out=at, in_=amat.ap())
            for g in range(1, NG):
                nc.sync.dma_start(out=xts[g], in_=xin[g])
            if with_bias:
                bias_bc = const.tile([128, T], F32)
                nc.gpsimd.dma_start(out=bias_bc, in_=bp.ap().to_broadcast([128, T]))

            # Pre-warm the PE HAM clock gate while the loads are in flight:
            # a memset-fed dummy matmul stream keeps PE continuously busy so
            # the real matmuls run at 2.4 GHz. gpsimd is free earliest, so it
            # does the memset.
            scratch = const.tile([128, 512], F16, tag="warm")
            nc.gpsimd.memset(scratch, 0.0)
            wps = psW.tile([128, 512], F32)
            for _ in range(N_WARM):
                nc.tensor.matmul(
                    wps,
                    lhsT=scratch[:, 0:128],
                    rhs=scratch,
                    start=True,
                    stop=True,
                )

            for g in range(NG):
                xt = xts[g]
                # h for the pair: [p=j%128, bi, jc(=node block m), t']
                h_sb = hbuf.tile([128, 2, 2, T], F16)
                for bi in range(2):
                    # both m chunks share one PSUM bank -> single wide copy
                    ph = psA.tile([128, 2, T], F32)  # [j%128, m, t']
                    for m in range(2):
                        nc.tensor.matmul(
                            ph[:, m],
                            lhsT=xt[:, 0, bi, bass.ts(m, 128)],
                            rhs=wt[:, 0],
                            start=True,
                            stop=False,
                        )
                        nc.tensor.matmul(
                            ph[:, m],
                            lhsT=xt[:, 1, bi, bass.ts(m, 128)],
                            rhs=wt[:, 1],
                            start=False,
                            stop=True,
                        )
                    # PSUM->SBUF cast copy, [128, 512] contiguous
                    nc.vector.tensor_copy(h_sb[:, bi], ph)
                ob = obuf.tile([128, 2, 2, T], F16)  # [p=i%128, m, bi, t']
                for m in range(2):
                    po = psB.tile([128, 2, T], F32)  # [i%128, bi, t'] one bank
                    nc.tensor.matmul(
                        po,
                        lhsT=at[:, 0, bass.ts(m, 128)],
                        rhs=h_sb[:, :, 0, :],
                        start=True,
                        stop=False,
                    )
                    nc.tensor.matmul(
                        po,
                        lhsT=at[:, 1, bass.ts(m, 128)],
                        rhs=h_sb[:, :, 1, :],
                        start=False,
                        stop=True,
                    )
                    last = g == NG - 1
                    if with_bias:
                        for bi in range(2):
                            nc.vector.tensor_add(
                                ob[:, m, bi, :], po[:, bi, :], bias_bc
                            )
                    elif last and m == 1:
                        # tail: DVE finishes its h copy earlier than ACT
                        nc.vector.tensor_copy(ob[:, m], po)
                    else:
                        # staging copy on ACT keeps DVE free for the h copies
                        nc.scalar.copy(out=ob[:, m], in_=po)
                    if last and m == 1:
                        # split the final store across both hwdge queues to
                        # overlap the two completion receipts
                        nc.sync.dma_start(
                            out=outp[g][:, 1, 0], in_=ob[:, 1, 0]
                        )
                        nc.scalar.dma_start(
                            out=outp[g][:, 1, 1], in_=ob[:, 1, 1]
                        )
                    else:
                        nc.sync.dma_start(out=outp[g][:, m], in_=ob[:, m])
    nc.finalize()
    return nc


def _ensure_axon_hooks_importable():
    """concourse's trace path hard-imports antenv.axon_hooks, which this
    image lacks. Provide the real ctypes-backed hook when possible, else a
    no-op, so BASS_TRACE=1 degrades gracefully instead of crashing."""
    try:
        import antenv.axon_hooks  # noqa: F401

        return
    except ImportError:
        pass
    try:
        import types

        import antenv

        mod = types.ModuleType("antenv.axon_hooks")
        state = {"h": None}
        mod.set_axon_ntff_profile_hook = lambda h: state.__setitem__("h", h)
        mod.get_axon_ntff_profile_hook = lambda: state["h"]
        sys.modules["antenv.axon_hooks"] = mod
        antenv.axon_hooks = mod
        try:
            from trn_agent_boot.trn_boot import _ntff_profile_via_ctypes

            hook = _ntff_profile_via_ctypes("/opt/axon/libaxon_pjrt.so")
            if hook is not None:
                mod.set_axon_ntff_profile_hook(hook)
        except Exception:
            pass
    except Exception:
        pass


def kernel(x, emb, W, b, logits, gumbel_u):
    global LAST_RESULT
    _ensure_axon_hooks_importable()
    from concourse.bass_utils import run_bass_kernel_spmd

    x = np.asarray(x, np.float32)
    W = np.asarray(W, np.float32)
    bias = np.ascontiguousarray(np.asarray(b, np.float32)).reshape(1, T)

    A = _graph_matrix(emb, logits, gumbel_u)
    # wmat [128, 2, 256]: W[c*128+p, t'];  amat: A^T likewise, partition-major
    wpack = np.ascontiguousarray(
        W.reshape(2, 128, T).transpose(1, 0, 2).astype(np.float16)
    )
    apack = np.ascontiguousarray(
        A.T.reshape(2, 128, N).transpose(1, 0, 2).astype(np.float16)
    )

    # xin [B/2 pairs, p, c, bi, n]: xT[b][t, n] split t = c*128+p, b = 2g+bi
    xT = x.transpose(0, 2, 1)  # [B, T, N]
    xpack = np.ascontiguousarray(
        xT.reshape(B // 2, 2, 2, 128, N).transpose(0, 3, 2, 1, 4).astype(np.float16)
    )

    with_bias = bool(np.any(bias))
    key = ("nc", with_bias)
    if key not in _CACHE:
        _CACHE[key] = _build_bass(with_bias)
    nc = _CACHE[key]

    in_maps = [
        {
            "xin": xpack[c * NG : (c + 1) * NG],
            "wmat": wpack,
            "amat": apack,
        }
        for c in range(NCORES)
    ]
    if with_bias:
        for m in in_maps:
            m["bias"] = bias
    # The first execution of a fresh NEFF occasionally trips a transient
    # NRT_EXEC_UNIT_UNRECOVERABLE; a straight retry reliably succeeds.
    last_exc = None
    for _attempt in range(3):
        try:
            res = run_bass_kernel_spmd(nc, in_maps, core_ids=list(range(NCORES)))
            break
        except Exception as e:  # noqa: BLE001
            last_exc = e
            import time as _time

            _time.sleep(2.0)
    else:
        raise last_exc
    LAST_RESULT = res
    # outp [NG, p, m, bi, t] -> out[2g+bi, m*128+p, t], upcast to f32
    out = np.empty((B, N, T), np.float32)
    for c in range(NCORES):
        ob = res.results[c]["outp"].astype(np.float32)  # [NG,128,2,2,T]
        ob = ob.transpose(0, 3, 2, 1, 4).reshape(BPC, N, T)  # [2g+bi, m*128+p, t]
        out[c * BPC : (c + 1) * BPC] = ob
    return out


# revision 5
# speedup vs baseline: 1.2135x; 1.1206x over previous
"""DynamicGraphEmbedding kernel for 8 Trainium2 NeuronCores.

The reference collapses algebraically:
  - deg[i] == K == 16 for every node (dst list is repeat(arange(N), K)),
    so gcn_norm edge weight ew == 1/16 for every edge.
  - straight-through gumbel gate is exactly y_hard in the forward pass,
    i.e. gate(e) = 1 iff argmax(softmax(logits[e] + g[e])) == 0.
  - therefore out[b] = A @ (x[b] @ W) + bias, with the dense [N, N] matrix
    A[i, j] = gate(i*N+j)/16 if j in topk_j[i] else 0.

Host (tiny, O(N^2)): build A from emb/logits/gumbel_u with the exact same
jax-on-CPU ops as the reference. Device (the memory-bound bulk): two chained
256^3 matmuls per batch element, data-parallel over batch across 8 cores.

All device traffic and matmuls are fp16 (halves HBM bytes vs f32 and enables
the PE fast-weight-load path); PSUM accumulation stays f32, output is
upcast to f32 on the host. A entries are gate/16, exact in fp16, so the only
quantization is x/W/h/out rounding: ~1e-3 relative, well under tolerance.

Schedule notes (trace-driven):
  - The NRT preamble pins every kernel instruction after ~7.1us; the first
    DMA completion semaphore can't fire before ~10.6us (issue + transfer +
    ~2.2us completion receipt). W and the first x pair ride in ONE DMA per
    hwdge queue ("head" params) so one semaphore gates the first matmul.
  - PE runs at 1.2 GHz until the HAM activity window (~3.4us) fills, and
    any idle gap re-throttles it (costs ~5us of re-ramp). A memset-fed
    warm-up matmul stream keeps PE continuously busy from ~7.7us until the
    head DMAs land, so all real matmuls run at 2.4 GHz.
  - PSUM->SBUF copies are 1x-rate (one PSUM read port): h copies ride DVE,
    out copies ride ACT, one [128,512] copy per PSUM bank.
"""

import sys

import numpy as np

if "/opt/trn_rl_repo" not in sys.path:
    sys.path.insert(0, "/opt/trn_rl_repo")

N, T, B, D, K = 256, 256, 64, 64, 16
NCORES = 8
BPC = B // NCORES  # batch elements per core
NG = BPC // 2  # batch pairs per core
N_WARM = 7  # PE HAM warm-up matmuls (512-free each, ~0.43us cold)

_CACHE = {}
LAST_RESULT = None  # BassKernelResults of the most recent run (for profiling)


def _graph_matrix(emb, logits, gumbel_u):
    """Dense [N, N] combined gate/topk/gcn-norm matrix A (host-side, tiny)."""
    try:
        import jax
        import jax.numpy as jnp

        cpu = jax.devices("cpu")[0]
        emb_j = jax.device_put(np.asarray(emb), cpu)
        logits_j = jax.device_put(np.asarray(logits), cpu)
        gu_j = jax.device_put(np.asarray(gumbel_u), cpu)
        nrm = jnp.linalg.norm(emb_j, axis=-1)
        cos = (emb_j @ emb_j.T) / (nrm[:, None] * nrm[None, :])
        _, topk_j = jax.lax.top_k(cos, K)
        g = -jnp.log(-jnp.log(gu_j))
        y_soft = jax.nn.softmax(logits_j + g, axis=-1)
        am = jnp.argmax(y_soft, axis=-1)
        topk = np.asarray(topk_j)
        gate_full = (np.asarray(am) == 0).astype(np.float32)
    except Exception:
        emb32 = np.asarray(emb, np.float32)
        nrm = np.sqrt((emb32 * emb32).sum(-1))
        cos = (emb32 @ emb32.T) / (nrm[:, None] * nrm[None, :])
        topk = np.argsort(-cos, axis=-1, kind="stable")[:, :K]
        lg = np.asarray(logits, np.float32) + np.float32(-1.0) * np.log(
            -np.log(np.asarray(gumbel_u, np.float32))
        )
        e = np.exp(lg - lg.max(-1, keepdims=True))
        y_soft = e / e.sum(-1, keepdims=True)
        gate_full = (np.argmax(y_soft, -1) == 0).astype(np.float32)
    rows = np.repeat(np.arange(N), K)
    cols = topk.reshape(-1)
    A = np.zeros((N, N), np.float32)
    A[rows, cols] = gate_full[rows * N + cols] * np.float32(0.0625)
    return A


def _build_bass(with_bias):
    """Per-core Bass graph: out[b] = A @ (x[b] @ W) [+ bias] for BPC batches.

    Host-packed fp16 layouts (contiguous per-partition runs, few big DMAs):
      head0  [128, 3, 256]        [p,0,t']=W[p,t'], [p,1+bi,n]=x[bi][n,p]
      head1  [128, 3, 256]        same with c=1: W[128+p], x[bi][n,128+p]
      amat   [128, 2, 256]        [p, c, i]:  A[i, c*128+p]  (A^T chunks)
      xin    [NG-1, 128, 2, 2, 256] pairs 1..NG-1: [g, p, c, bi, n]
      bias   [1, 256]             (only when with_bias)
      outp   [NG, 128, 2, 2, 256] [g, p, m, bi, t] = out[2g+bi][m*128+p, t]
    """
    import concourse.bass as bass
    import concourse.mybir as mybir
    from concourse import bacc
    from concourse.tile import TileContext

    F32 = mybir.dt.float32
    F16 = mybir.dt.float16

    nc = bacc.Bacc()
    head0 = nc.declare_dram_parameter("head0", [128, 3, 256], F16, isOutput=False)
    head1 = nc.declare_dram_parameter("head1", [128, 3, 256], F16, isOutput=False)
    amat = nc.declare_dram_parameter("amat", [128, 2, 256], F16, isOutput=False)
    xin = nc.declare_dram_parameter(
        "xin", [NG - 1, 128, 2, 2, N], F16, isOutput=False
    )
    if with_bias:
        bp = nc.declare_dram_parameter("bias", [1, T], F32, isOutput=False)
    outp = nc.declare_dram_parameter("outp", [NG, 128, 2, 2, T], F16, isOutput=True)

    with TileContext(nc) as tc:
        with (
            tc.tile_pool(name="const", bufs=1) as const,
            tc.tile_pool(name="xpool", bufs=3) as xpool,
            tc.tile_pool(name="hbuf", bufs=3) as hbuf,
            tc.tile_pool(name="obuf", bufs=4) as obuf,
            tc.tile_pool(name="psA", bufs=4, space="PSUM") as psA,
            tc.tile_pool(name="psB", bufs=3, space="PSUM") as psB,
            tc.tile_pool(name="psW", bufs=1, space="PSUM") as psW,
        ):
            # One DMA per hwdge queue delivers W chunk + x pair-0 chunk, so a
            # single completion semaphore gates the first real matmul.
            ht0 = const.tile([128, 3, 256], F16)
            ht1 = const.tile([128, 3, 256], F16)
            at = const.tile([128, 2, 256], F16)
            nc.sync.dma_start(out=ht0, in_=head0.ap())
            nc.scalar.dma_start(out=ht1, in_=head1.ap())
            nc.scalar.dma_start(out=at, in_=amat.ap())
            xts = []
            for g in range(1, NG):
                xt = xpool.tile([128, 2, 2, N], F16, tag=f"xt{g}")  # [p,c,bi,n]
                nc.sync.dma_start(out=xt, in_=xin[g - 1])
                xts.append(xt)
            if with_bias:
                bias_bc = const.tile([128, T], F32)
                nc.gpsimd.dma_start(out=bias_bc, in_=bp.ap().to_broadcast([128, T]))

            # HAM warm-up: keep PE continuously busy from ~7.7us until the
            # head DMAs land (~10.7us) so real matmuls run at 2.4 GHz.
            scratch = const.tile([128, 512], F16, tag="warm")
            nc.gpsimd.memset(scratch, 0.0)
            wps = psW.tile([128, 512], F32)
            for _ in range(N_WARM):
                nc.tensor.matmul(
                    wps,
                    lhsT=scratch[:, 0:128],
                    rhs=scratch,
                    start=True,
                    stop=True,
                )

            def xap(g, c, bi, mslice):
                """lhsT chunk [128, 128] for pair g, contraction chunk c."""
                if g == 0:
                    ht = ht0 if c == 0 else ht1
                    return ht[:, 1 + bi, mslice]
                return xts[g - 1][:, c, bi, mslice]

            for g in range(NG):
                # h for the pair: [p=j%128, bi, jc(=node block m), t']
                h_sb = hbuf.tile([128, 2, 2, T], F16)
                for bi in range(2):
                    # both m chunks share one PSUM bank -> single wide copy
                    ph = psA.tile([128, 2, T], F32)  # [j%128, m, t']
                    for m in range(2):
                        nc.tensor.matmul(
                            ph[:, m],
                            lhsT=xap(g, 0, bi, bass.ts(m, 128)),
                            rhs=ht0[:, 0],
                            start=True,
                            stop=False,
                        )
                        nc.tensor.matmul(
                            ph[:, m],
                            lhsT=xap(g, 1, bi, bass.ts(m, 128)),
                            rhs=ht1[:, 0],
                            start=False,
                            stop=True,
                        )
                    # PSUM->SBUF cast copy, [128, 512] contiguous
                    nc.vector.tensor_copy(h_sb[:, bi], ph)
                ob = obuf.tile([128, 2, 2, T], F16)  # [p=i%128, m, bi, t']
                for m in range(2):
                    po = psB.tile([128, 2, T], F32)  # [i%128, bi, t'] one bank
                    nc.tensor.matmul(
                        po,
                        lhsT=at[:, 0, bass.ts(m, 128)],
                        rhs=h_sb[:, :, 0, :],
                        start=True,
                        stop=False,
                    )
                    nc.tensor.matmul(
                        po,
                        lhsT=at[:, 1, bass.ts(m, 128)],
                        rhs=h_sb[:, :, 1, :],
                        start=False,
                        stop=True,
                    )
                    last = g == NG - 1
                    if with_bias:
                        for bi in range(2):
                            nc.vector.tensor_add(
                                ob[:, m, bi, :], po[:, bi, :], bias_bc
                            )
                        nc.sync.dma_start(out=outp[g][:, m], in_=ob[:, m])
                    elif last and m == 1:
                        # tail: split the final copy across DVE + ACT and the
                        # final stores across both hwdge queues so the two
                        # completion receipts overlap
                        nc.vector.tensor_copy(ob[:, 1, 0], po[:, 0])
                        nc.scalar.copy(out=ob[:, 1, 1], in_=po[:, 1])
                        nc.sync.dma_start(out=outp[g][:, 1, 0], in_=ob[:, 1, 0])
                        nc.scalar.dma_start(out=outp[g][:, 1, 1], in_=ob[:, 1, 1])
                    else:
                        # staging copy on ACT keeps DVE free for the h copies
                        nc.scalar.copy(out=ob[:, m], in_=po)
                        nc.sync.dma_start(out=outp[g][:, m], in_=ob[:, m])
    nc.finalize()
    return nc


def _ensure_axon_hooks_importable():
    """concourse's trace path hard-imports antenv.axon_hooks, which this
    image lacks. Provide the real ctypes-backed hook when possible, else a
    no-op, so BASS_TRACE=1 degrades gracefully instead of crashing."""
    try:
        import antenv.axon_hooks  # noqa: F401

        return
    except ImportError:
        pass
    try:
        import types

        import antenv

        mod = types.ModuleType("antenv.axon_hooks")
        state = {"h": None}
        mod.set_axon_ntff_profile_hook = lambda h: state.__setitem__("h", h)
        mod.get_axon_ntff_profile_hook = lambda: state["h"]
        sys.modules["antenv.axon_hooks"] = mod
        antenv.axon_hooks = mod
        try:
            from trn_agent_boot.trn_boot import _ntff_profile_via_ctypes

            hook = _ntff_profile_via_ctypes("/opt/axon/libaxon_pjrt.so")
            if hook is not None:
                mod.set_axon_ntff_profile_hook(hook)
        except Exception:
            pass
    except Exception:
        pass


def kernel(x, emb, W, b, logits, gumbel_u):
    global LAST_RESULT
    _ensure_axon_hooks_importable()
    from concourse.bass_utils import run_bass_kernel_spmd

    x = np.asarray(x, np.float32)
    W = np.asarray(W, np.float32)
    bias = np.ascontiguousarray(np.asarray(b, np.float32)).reshape(1, T)

    A = _graph_matrix(emb, logits, gumbel_u)
    W16 = W.astype(np.float16)  # [t, t'], t = c*128 + p
    A16 = np.ascontiguousarray(A.T).astype(np.float16)  # [j, i], j = c*128 + p
    apack = np.ascontiguousarray(A16.reshape(2, 128, N).transpose(1, 0, 2))

    # xin [B/2 pairs, p, c, bi, n]: xT[b][t, n] split t = c*128+p, b = 2g+bi
    xT = x.transpose(0, 2, 1).astype(np.float16)  # [B, T, N]
    xpack = np.ascontiguousarray(
        xT.reshape(B // 2, 2, 2, 128, N).transpose(0, 3, 2, 1, 4)
    )  # [g, p, c, bi, n]

    with_bias = bool(np.any(bias))
    key = ("nc", with_bias)
    if key not in _CACHE:
        _CACHE[key] = _build_bass(with_bias)
    nc = _CACHE[key]

    # head{c} [128, 3, 256] per core: [:,0]=W[c*128+p], [:,1+bi]=x[2gc+bi] c
    wr = W16.reshape(2, 128, T)  # [c, p, t']
    in_maps = []
    for c in range(NCORES):
        xg = xpack[c * NG : (c + 1) * NG]  # [NG, p, c, bi, n]
        h0 = np.concatenate([wr[0][:, None, :], xg[0][:, 0]], axis=1)
        h1 = np.concatenate([wr[1][:, None, :], xg[0][:, 1]], axis=1)
        in_maps.append(
            {
                "head0": np.ascontiguousarray(h0),
                "head1": np.ascontiguousarray(h1),
                "amat": apack,
                "xin": xg[1:],
            }
        )
    if with_bias:
        for m in in_maps:
            m["bias"] = bias
    # The first execution of a fresh NEFF occasionally trips a transient
    # NRT_EXEC_UNIT_UNRECOVERABLE; a straight retry reliably succeeds.
    last_exc = None
    for _attempt in range(3):
        try:
            res = run_bass_kernel_spmd(nc, in_maps, core_ids=list(range(NCORES)))
            break
        except Exception as e:  # noqa: BLE001
            last_exc = e
            import time as _time

            _time.sleep(2.0)
    else:
        raise last_exc
    LAST_RESULT = res
    # outp [NG, p, m, bi, t] -> out[2g+bi, m*128+p, t], upcast to f32
    out = np.empty((B, N, T), np.float32)
    for c in range(NCORES):
        ob = res.results[c]["outp"].astype(np.float32)  # [NG,128,2,2,T]
        ob = ob.transpose(0, 3, 2, 1, 4).reshape(BPC, N, T)  # [2g+bi, m*128+p, t]
        out[c * BPC : (c + 1) * BPC] = ob
    return out


# revision 6
# speedup vs baseline: 1.2212x; 1.0063x over previous
"""DynamicGraphEmbedding kernel for 8 Trainium2 NeuronCores.

The reference collapses algebraically:
  - deg[i] == K == 16 for every node (dst list is repeat(arange(N), K)),
    so gcn_norm edge weight ew == 1/16 for every edge.
  - straight-through gumbel gate is exactly y_hard in the forward pass,
    i.e. gate(e) = 1 iff argmax(softmax(logits[e] + g[e])) == 0.
  - therefore out[b] = A @ (x[b] @ W) + bias, with the dense [N, N] matrix
    A[i, j] = gate(i*N+j)/16 if j in topk_j[i] else 0.

Host (tiny, O(N^2)): build A from emb/logits/gumbel_u with the exact same
jax-on-CPU ops as the reference. Device (the memory-bound bulk): two chained
256^3 matmuls per batch element, data-parallel over batch across 8 cores.

All device traffic and matmuls are fp16 (halves HBM bytes vs f32 and enables
the PE fast-weight-load path); PSUM accumulation stays f32, output is
upcast to f32 on the host. A entries are gate/16, exact in fp16, so the only
quantization is x/W/h/out rounding: ~1e-3 relative, well under tolerance.

Schedule notes (trace-driven):
  - The NRT preamble pins every kernel instruction after ~7.1us; the first
    DMA completion semaphore can't fire before ~10.6us (issue + transfer +
    ~2.2us completion receipt). W and the first x pair ride in ONE DMA per
    hwdge queue ("head" params) so one semaphore gates the first matmul.
  - PE runs at 1.2 GHz until the HAM activity window (~3.4us) fills, and
    any idle gap re-throttles it (costs ~5us of re-ramp). A memset-fed
    warm-up matmul stream keeps PE continuously busy from ~7.7us until the
    head DMAs land, so all real matmuls run at 2.4 GHz.
  - PSUM->SBUF copies are 1x-rate (one PSUM read port): h copies ride DVE,
    out copies ride ACT, one [128,512] copy per PSUM bank.
"""

import sys

import numpy as np

if "/opt/trn_rl_repo" not in sys.path:
    sys.path.insert(0, "/opt/trn_rl_repo")

N, T, B, D, K = 256, 256, 64, 64, 16
NCORES = 8
BPC = B // NCORES  # batch elements per core
NG = BPC // 2  # batch pairs per core
N_WARM = 7  # PE HAM warm-up matmuls (512-free each, ~0.43us cold)

_CACHE = {}
LAST_RESULT = None  # BassKernelResults of the most recent run (for profiling)


def _graph_matrix(emb, logits, gumbel_u):
    """Dense [N, N] combined gate/topk/gcn-norm matrix A (host-side, tiny)."""
    try:
        import jax
        import jax.numpy as jnp

        cpu = jax.devices("cpu")[0]
        emb_j = jax.device_put(np.asarray(emb), cpu)
        logits_j = jax.device_put(np.asarray(logits), cpu)
        gu_j = jax.device_put(np.asarray(gumbel_u), cpu)
        nrm = jnp.linalg.norm(emb_j, axis=-1)
        cos = (emb_j @ emb_j.T) / (nrm[:, None] * nrm[None, :])
        _, topk_j = jax.lax.top_k(cos, K)
        g = -jnp.log(-jnp.log(gu_j))
        y_soft = jax.nn.softmax(logits_j + g, axis=-1)
        am = jnp.argmax(y_soft, axis=-1)
        topk = np.asarray(topk_j)
        gate_full = (np.asarray(am) == 0).astype(np.float32)
    except Exception:
        emb32 = np.asarray(emb, np.float32)
        nrm = np.sqrt((emb32 * emb32).sum(-1))
        cos = (emb32 @ emb32.T) / (nrm[:, None] * nrm[None, :])
        topk = np.argsort(-cos, axis=-1, kind="stable")[:, :K]
        lg = np.asarray(logits, np.float32) + np.float32(-1.0) * np.log(
            -np.log(np.asarray(gumbel_u, np.float32))
        )
        e = np.exp(lg - lg.max(-1, keepdims=True))
        y_soft = e / e.sum(-1, keepdims=True)
        gate_full = (np.argmax(y_soft, -1) == 0).astype(np.float32)
    rows = np.repeat(np.arange(N), K)
    cols = topk.reshape(-1)
    A = np.zeros((N, N), np.float32)
    A[rows, cols] = gate_full[rows * N + cols] * np.float32(0.0625)
    return A


def _build_bass(with_bias):
    """Per-core Bass graph: out[b] = A @ (x[b] @ W) [+ bias] for BPC batches.

    Host-packed fp16 layouts (contiguous per-partition runs, few big DMAs):
      head0  [128, 3, 256]        [p,0,t']=W[p,t'], [p,1+bi,n]=x[bi][n,p]
      head1  [128, 3, 256]        same with c=1: W[128+p], x[bi][n,128+p]
      amat   [128, 2, 256]        [p, c, i]:  A[i, c*128+p]  (A^T chunks)
      xin    [NG-1, 128, 2, 2, 256] pairs 1..NG-1: [g, p, c, bi, n]
      bias   [1, 256]             (only when with_bias)
      outp   [NG, 128, 2, 2, 256] [g, p, m, bi, t] = out[2g+bi][m*128+p, t]
    """
    import concourse.bass as bass
    import concourse.mybir as mybir
    from concourse import bacc
    from concourse.tile import TileContext

    F32 = mybir.dt.float32
    F16 = mybir.dt.float16

    nc = bacc.Bacc()
    head0 = nc.declare_dram_parameter("head0", [128, 3, 256], F16, isOutput=False)
    head1 = nc.declare_dram_parameter("head1", [128, 3, 256], F16, isOutput=False)
    amat = nc.declare_dram_parameter("amat", [128, 2, 256], F16, isOutput=False)
    xin = nc.declare_dram_parameter(
        "xin", [NG - 1, 128, 2, 2, N], F16, isOutput=False
    )
    if with_bias:
        bp = nc.declare_dram_parameter("bias", [1, T], F32, isOutput=False)
    outp = nc.declare_dram_parameter("outp", [NG, 128, 2, 2, T], F16, isOutput=True)

    with TileContext(nc) as tc:
        with (
            tc.tile_pool(name="const", bufs=1) as const,
            tc.tile_pool(name="xpool", bufs=3) as xpool,
            tc.tile_pool(name="hbuf", bufs=3) as hbuf,
            tc.tile_pool(name="obuf", bufs=4) as obuf,
            tc.tile_pool(name="psA", bufs=4, space="PSUM") as psA,
            tc.tile_pool(name="psB", bufs=3, space="PSUM") as psB,
            tc.tile_pool(name="psW", bufs=1, space="PSUM") as psW,
        ):
            # One DMA per hwdge queue delivers W chunk + x pair-0 chunk, so a
            # single completion semaphore gates the first real matmul.
            ht0 = const.tile([128, 3, 256], F16)
            ht1 = const.tile([128, 3, 256], F16)
            at = const.tile([128, 2, 256], F16)
            nc.sync.dma_start(out=ht0, in_=head0.ap())
            nc.scalar.dma_start(out=ht1, in_=head1.ap())
            nc.scalar.dma_start(out=at, in_=amat.ap())
            xts = []
            for g in range(1, NG):
                xt = xpool.tile([128, 2, 2, N], F16, tag=f"xt{g}")  # [p,c,bi,n]
                nc.sync.dma_start(out=xt, in_=xin[g - 1])
                xts.append(xt)
            if with_bias:
                bias_bc = const.tile([128, T], F32)
                nc.gpsimd.dma_start(out=bias_bc, in_=bp.ap().to_broadcast([128, T]))

            # HAM warm-up: keep PE continuously busy from ~7.7us until the
            # head DMAs land (~10.7us) so real matmuls run at 2.4 GHz.
            scratch = const.tile([128, 512], F16, tag="warm")
            nc.gpsimd.memset(scratch, 0.0)
            wps = psW.tile([128, 512], F32)
            for _ in range(N_WARM):
                nc.tensor.matmul(
                    wps,
                    lhsT=scratch[:, 0:128],
                    rhs=scratch,
                    start=True,
                    stop=True,
                )

            def xap(g, c, bi, mslice):
                """lhsT chunk [128, 128] for pair g, contraction chunk c."""
                if g == 0:
                    ht = ht0 if c == 0 else ht1
                    return ht[:, 1 + bi, mslice]
                return xts[g - 1][:, c, bi, mslice]

            for g in range(NG):
                # h for the pair: [p=j%128, bi, jc(=node block m), t']
                h_sb = hbuf.tile([128, 2, 2, T], F16)
                for bi in range(2):
                    # both m chunks share one PSUM bank -> single wide copy
                    ph = psA.tile([128, 2, T], F32)  # [j%128, m, t']
                    for m in range(2):
                        nc.tensor.matmul(
                            ph[:, m],
                            lhsT=xap(g, 0, bi, bass.ts(m, 128)),
                            rhs=ht0[:, 0],
                            start=True,
                            stop=False,
                        )
                        nc.tensor.matmul(
                            ph[:, m],
                            lhsT=xap(g, 1, bi, bass.ts(m, 128)),
                            rhs=ht1[:, 0],
                            start=False,
                            stop=True,
                        )
                    # PSUM->SBUF cast copy, [128, 512] contiguous
                    nc.vector.tensor_copy(h_sb[:, bi], ph)
                ob = obuf.tile([128, 2, 2, T], F16)  # [p=i%128, m, bi, t']
                for m in range(2):
                    po = psB.tile([128, 2, T], F32)  # [i%128, bi, t'] one bank
                    nc.tensor.matmul(
                        po,
                        lhsT=at[:, 0, bass.ts(m, 128)],
                        rhs=h_sb[:, :, 0, :],
                        start=True,
                        stop=False,
                    )
                    nc.tensor.matmul(
                        po,
                        lhsT=at[:, 1, bass.ts(m, 128)],
                        rhs=h_sb[:, :, 1, :],
                        start=False,
                        stop=True,
                    )
                    last = g == NG - 1
                    if with_bias:
                        for bi in range(2):
                            nc.vector.tensor_add(
                                ob[:, m, bi, :], po[:, bi, :], bias_bc
                            )
                        nc.sync.dma_start(out=outp[g][:, m], in_=ob[:, m])
                    elif last:
                        # tail: DVE is idle by now while ACT still drains the
                        # previous pair's copies -> m0 on DVE, m1 on ACT, one
                        # store per hwdge queue so the completion receipts
                        # overlap
                        if m == 0:
                            nc.vector.tensor_copy(ob[:, 0], po)
                            nc.sync.dma_start(out=outp[g][:, 0], in_=ob[:, 0])
                        else:
                            nc.scalar.copy(out=ob[:, 1], in_=po)
                            nc.scalar.dma_start(out=outp[g][:, 1], in_=ob[:, 1])
                    else:
                        # staging copy on ACT keeps DVE free for the h copies
                        nc.scalar.copy(out=ob[:, m], in_=po)
                        nc.sync.dma_start(out=outp[g][:, m], in_=ob[:, m])
    nc.finalize()
    return nc


def _ensure_axon_hooks_importable():
    """concourse's trace path hard-imports antenv.axon_hooks, which this
    image lacks. Provide the real ctypes-backed hook when possible, else a
    no-op, so BASS_TRACE=1 degrades gracefully instead of crashing."""
    try:
        import antenv.axon_hooks  # noqa: F401

        return
    except ImportError:
        pass
    try:
        import types

        import antenv

        mod = types.ModuleType("antenv.axon_hooks")
        state = {"h": None}
        mod.set_axon_ntff_profile_hook = lambda h: state.__setitem__("h", h)
        mod.get_axon_ntff_profile_hook = lambda: state["h"]
        sys.modules["antenv.axon_hooks"] = mod
        antenv.axon_hooks = mod
        try:
            from trn_agent_boot.trn_boot import _ntff_profile_via_ctypes

            hook = _ntff_profile_via_ctypes("/opt/axon/libaxon_pjrt.so")
            if hook is not None:
                mod.set_axon_ntff_profile_hook(hook)
        except Exception:
            pass
    except Exception:
        pass


def kernel(x, emb, W, b, logits, gumbel_u):
    global LAST_RESULT
    _ensure_axon_hooks_importable()
    from concourse.bass_utils import run_bass_kernel_spmd

    x = np.asarray(x, np.float32)
    W = np.asarray(W, np.float32)
    bias = np.ascontiguousarray(np.asarray(b, np.float32)).reshape(1, T)

    A = _graph_matrix(emb, logits, gumbel_u)
    W16 = W.astype(np.float16)  # [t, t'], t = c*128 + p
    A16 = np.ascontiguousarray(A.T).astype(np.float16)  # [j, i], j = c*128 + p
    apack = np.ascontiguousarray(A16.reshape(2, 128, N).transpose(1, 0, 2))

    # xin [B/2 pairs, p, c, bi, n]: xT[b][t, n] split t = c*128+p, b = 2g+bi
    xT = x.transpose(0, 2, 1).astype(np.float16)  # [B, T, N]
    xpack = np.ascontiguousarray(
        xT.reshape(B // 2, 2, 2, 128, N).transpose(0, 3, 2, 1, 4)
    )  # [g, p, c, bi, n]

    with_bias = bool(np.any(bias))
    key = ("nc", with_bias)
    if key not in _CACHE:
        _CACHE[key] = _build_bass(with_bias)
    nc = _CACHE[key]

    # head{c} [128, 3, 256] per core: [:,0]=W[c*128+p], [:,1+bi]=x[2gc+bi] c
    wr = W16.reshape(2, 128, T)  # [c, p, t']
    in_maps = []
    for c in range(NCORES):
        xg = xpack[c * NG : (c + 1) * NG]  # [NG, p, c, bi, n]
        h0 = np.concatenate([wr[0][:, None, :], xg[0][:, 0]], axis=1)
        h1 = np.concatenate([wr[1][:, None, :], xg[0][:, 1]], axis=1)
        in_maps.append(
            {
                "head0": np.ascontiguousarray(h0),
                "head1": np.ascontiguousarray(h1),
                "amat": apack,
                "xin": xg[1:],
            }
        )
    if with_bias:
        for m in in_maps:
            m["bias"] = bias
    # The first execution of a fresh NEFF occasionally trips a transient
    # NRT_EXEC_UNIT_UNRECOVERABLE; a straight retry reliably succeeds.
    last_exc = None
    for _attempt in range(3):
        try:
            res = run_bass_kernel_spmd(nc, in_maps, core_ids=list(range(NCORES)))
            break
        except Exception as e:  # noqa: BLE001
            last_exc = e
            import time as _time

            _time.sleep(2.0)
    else:
        raise last_exc
    LAST_RESULT = res
    # outp [NG, p, m, bi, t] -> out[2g+bi, m*128+p, t], upcast to f32
    out = np.empty((B, N, T), np.float32)
    for c in range(NCORES):
        ob = res.results[c]["outp"].astype(np.float32)  # [NG,128,2,2,T]
        ob = ob.transpose(0, 3, 2, 1, 4).reshape(BPC, N, T)  # [2g+bi, m*128+p, t]
        out[c * BPC : (c + 1) * BPC] = ob
    return out
